# revision 1
# baseline (speedup 1.0000x reference)
"""Cross-attention Trainium2 kernel (Bass/Tile), SPMD over 8 NeuronCores.

Problem: b=8, i=j=2048, query/context dim 512, inner dim 256.
Sharding: data-parallel over batch — one batch element per core, no
collectives. Each core computes, for its batch element:

    q = x @ Wq ; k = ctx @ Wk ; v = ctx @ Wv
    sim = (q @ k^T) * d^-0.5 ; attn = softmax_j(sim) masked on j
    out = attn @ v ; y = out @ Wo + bo + x

Per-core dataflow (all matmuls bf16 with fp32 PSUM accumulation):
  1. Cast x and ctx to bf16, then PE-transpose (1 cyc/row) into
     k-major layout (xT, ctxT).
  2. qT = Wq^T @ xT, kT = Wk^T @ ctxT (d-major); v = ctxT^T @ Wv
     (j-major natural). The mask is folded into the v copy as a
     per-partition (per-j) scale, zeroing rows of v for masked j.
  3. Per 512-col i-block: simT[j, i] = kT^T @ qT -> PSUM, computed
     TRANSPOSED so the exp on ScalarE (values are bounded, so no max
     subtraction is needed) writes the j-major layout the PV matmul
     needs directly — no 128x128 attn transposes or extra copies.
  4. outT = v^T @ attnT accumulated over all j; softmax denominators
     come from a mask-weighted ones-row matmul on PE
     (denom = mask^T @ exp, which is also the correctly-masked sum),
     transposed to i-major via tiny K=1 matmuls.
  5. y = outT^T @ Wo, scaled by the softmax reciprocal per row (row
     scaling commutes with the right-multiply by Wo), plus x and bo.
"""

import sys

import numpy as np

if "/opt/trn_rl_repo" not in sys.path:
    sys.path.insert(0, "/opt/trn_rl_repo")

_P = 128          # partitions
_B = 8            # batch == number of cores
_I = 2048         # query sequence length
_J = 2048         # context sequence length
_K = 512          # query/context feature dim
_D = 256          # inner dim
_NB = 512         # matmul free-dim block
_KT = _K // _P    # 4 contraction tiles for projections
_DT = _D // _P    # 2 inner-dim tiles
_IT = _I // _P    # 16 i tiles
_JT = _J // _P    # 16 j tiles
_SCALE = float(_D) ** -0.5

_CACHE = {}


def _split_multi_waits(nc, limits):
    """Walrus in this container rejects instructions carrying more sem
    waits than its per-template slot count (e.g. Drain allows 1). Move
    excess waits onto wait-only Drain carriers on the same engine,
    inserted just before the instruction — semantically identical."""
    from concourse import mybir

    n_split = 0
    for func in nc.m.functions:
        for block in func.blocks:
            out = []
            for inst in block.instructions:
                si = inst.sync_info
                maxw = limits.get(type(inst).__name__, limits.get("*"))
                if (
                    maxw is not None
                    and si is not None
                    and si.on_wait
                    and len(si.on_wait) > maxw
                ):
                    waits = list(si.on_wait)
                    keep, rest = waits[:maxw], waits[maxw:]
                    for i in range(0, len(rest), 1):
                        car = mybir.InstDrain(
                            name=f"I-waitcar-{nc.next_id()}", ins=[], outs=[]
                        )
                        car.engine = inst.engine
                        car.sync_info = mybir.SyncInfo(
                            on_wait=[rest[i]], on_update=[]
                        )
                        nc.register_instruction(car)
                        out.append(car)
                        n_split += 1
                    inst.sync_info = mybir.SyncInfo(
                        on_wait=keep, on_update=list(si.on_update or [])
                    )
                out.append(inst)
            block.instructions = out
    return n_split


def _build_nc(repeat=1):
    import concourse.bass as bass
    import concourse.tile as tile
    from concourse import mybir
    from concourse.masks import make_identity

    dt = mybir.dt
    Alu = mybir.AluOpType
    Act = mybir.ActivationFunctionType

    nc = bass.Bass("TRN2", target_bir_lowering=False)

    x_d = nc.dram_tensor("x", [_I, _K], dt.float32, kind="ExternalInput")
    c_d = nc.dram_tensor("context", [_J, _K], dt.float32, kind="ExternalInput")
    m_d = nc.dram_tensor("mask", [_J], dt.uint8, kind="ExternalInput")
    wq_d = nc.dram_tensor("Wq", [_K, _D], dt.float32, kind="ExternalInput")
    wk_d = nc.dram_tensor("Wk", [_K, _D], dt.float32, kind="ExternalInput")
    wv_d = nc.dram_tensor("Wv", [_K, _D], dt.float32, kind="ExternalInput")
    wo_d = nc.dram_tensor("Wo", [_D, _K], dt.float32, kind="ExternalInput")
    bo_d = nc.dram_tensor("bo", [_K], dt.float32, kind="ExternalInput")
    y_d = nc.dram_tensor("out", [_I, _K], dt.float32, kind="ExternalOutput")

    with tile.TileContext(nc) as tc:
        with (
            tc.tile_pool(name="persist", bufs=1) as persist,
            tc.tile_pool(name="stage", bufs=3) as stage,
            tc.tile_pool(name="small", bufs=4) as small,
            tc.tile_pool(name="attnT", bufs=2) as attntp,
            tc.tile_pool(name="yout", bufs=3) as youtp,
            tc.tile_pool(name="psmm", bufs=2, space="PSUM") as psmm,
            tc.tile_pool(name="psacc", bufs=2, space="PSUM") as psacc,
            tc.tile_pool(name="pstr", bufs=3, space="PSUM") as pstr,
        ):
            # ---------------- constants / weights ----------------
            ident_b = persist.tile([_P, _P], dt.bfloat16, tag="identb")
            make_identity(nc, ident_b)
            one_one = persist.tile([1, 1], dt.float32, tag="one_one")
            nc.vector.memset(one_one, 1.0)

            # ---------------- persistent activations ----------------
            x_nat = persist.tile([_P, _IT, _K], dt.float32, tag="xnat")
            xT = persist.tile([_P, _KT, _I], dt.bfloat16, tag="xT")
            cT = persist.tile([_P, _KT, _J], dt.bfloat16, tag="cT")
            qT = persist.tile([_P, _DT, _I], dt.bfloat16, tag="qT")
            kT = persist.tile([_P, _DT, _J], dt.bfloat16, tag="kT")
            v = persist.tile([_P, _JT, _D], dt.bfloat16, tag="v")
            oT = persist.tile([_P, _DT, _I], dt.bfloat16, tag="oT")
            recips = persist.tile([_P, _IT], dt.float32, tag="recips")

            wq = persist.tile([_P, _KT, _D], dt.bfloat16, tag="wq")
            wk = persist.tile([_P, _KT, _D], dt.bfloat16, tag="wk")
            wv = persist.tile([_P, _KT, _D], dt.bfloat16, tag="wv")
            wo = persist.tile([_P, _DT, _K], dt.bfloat16, tag="wo")
            mask01 = persist.tile([_P, _JT], dt.float32, tag="mask01")
            mask01b = persist.tile([_P, _JT], dt.bfloat16, tag="mask01b")
            bo_bc = persist.tile([_P, _K], dt.float32, tag="bobc")

            # `repeat` > 1 chains extra full iterations for timing
            # calibration; WAW deps on the persistent tiles serialize
            # them so (t_N - t_1)/(N-1) approximates one iteration.
            for _rep in range(repeat):
                # ---------------- input loading + transposes ----------------
                # DMA issue order matters: SP's HWDGE is a FIFO. Load the
                # first ctx tiles before the weights so the PE starts
                # transposing immediately; x rides behind.
                def load_weights_early():
                    for w_dram, w_sb, nt in ((wk_d, wk, _KT), (wv_d, wv, _KT)):
                        ws = stage.tile([_P, nt, _D], dt.float32, tag="wstage",
                                        name=f"ws_{w_sb.name}")
                        nc.sync.dma_start(
                            out=ws, in_=w_dram[:].rearrange("(t p) d -> p t d", p=_P)
                        )
                        nc.vector.tensor_copy(out=w_sb, in_=ws)
                    msk8 = small.tile([_P, _JT], dt.uint8, tag="msk8")
                    nc.sync.dma_start(
                        out=msk8, in_=m_d[:].rearrange("(t p) -> p t", p=_P)
                    )
                    nc.vector.tensor_copy(out=mask01, in_=msk8)
                    nc.vector.tensor_copy(out=mask01b, in_=mask01)

                def load_weights_late():
                    for w_dram, w_sb in ((wq_d, wq),):
                        ws = stage.tile([_P, _KT, _D], dt.float32, tag="wstage",
                                        name=f"ws_{w_sb.name}")
                        nc.sync.dma_start(
                            out=ws, in_=w_dram[:].rearrange("(t p) d -> p t d", p=_P)
                        )
                        nc.vector.tensor_copy(out=w_sb, in_=ws)
                    ws = stage.tile([_P, _DT, _K], dt.float32, tag="wstage",
                                    name="ws_wo")
                    nc.sync.dma_start(
                        out=ws, in_=wo_d[:].rearrange("(t p) k -> p t k", p=_P)
                    )
                    nc.vector.tensor_copy(out=wo, in_=ws)
                    bo_ap = bo_d[:]
                    nc.sync.dma_start(
                        out=bo_bc,
                        in_=bass.AP(
                            tensor=bo_ap.tensor, offset=bo_ap.offset,
                            ap=[[0, _P], bo_ap.ap[0]],
                        ),
                    )

                # ctx -> ctxT (bf16, k on partitions) via PE transposes.
                # 1MiB DMAs (4 row-tiles each) keep the feed rate above
                # the PE's transpose+projection consumption rate.
                for g in range(_JT // 4):
                    cn = stage.tile([_P, 4, _K], dt.float32, tag="cnat",
                                    bufs=3)
                    # Half-group (512KB) DMAs: the casts/transposes for
                    # the first two tiles start as soon as the first half
                    # lands. The very first tile gets its own 256KB DMA
                    # so the PE's first transpose starts earliest.
                    if g == 0:
                        splits = ((0, 1), (1, 1), (2, 2))
                    else:
                        splits = ((0, 2), (2, 2))
                    for o, n in splits:
                        r0 = (g * 4 + o) * _P
                        nc.sync.dma_start(
                            out=cn[:, o:o + n, :],
                            in_=c_d[r0:r0 + n * _P, :].rearrange(
                                "(t p) k -> p t k", p=_P
                            ),
                        )
                    if g == 1:
                        load_weights_early()
                    for tt in range(4):
                        jt = g * 4 + tt
                        # bf16 transposes run at 1 cyc/row (vs 2 for f32);
                        # the pre-cast rides on ScalarE.
                        cnb = stage.tile([_P, _K], dt.bfloat16, tag="cnb",
                                         bufs=4)
                        nc.vector.tensor_copy(out=cnb, in_=cn[:, tt, :])
                        for kt in range(_KT):
                            tr = pstr.tile([_P, _P], dt.bfloat16, tag="tr")
                            nc.tensor.transpose(
                                tr, cnb[:, kt * _P:(kt + 1) * _P], ident_b
                            )
                            dst = cT[:, kt, jt * _P:(jt + 1) * _P]
                            if kt % 2 == 0:
                                # DVE moves bits; uint32 view halves the
                                # element count (bf16 pair per lane).
                                nc.vector.tensor_copy(
                                    out=dst.bitcast(dt.uint32),
                                    in_=tr[:].bitcast(dt.uint32),
                                )
                            else:
                                nc.scalar.copy(out=dst, in_=tr)

                # kT projection: kT[d, j] = Wk^T @ ctxT
                for dh in range(_DT):
                    for jb in range(_J // _NB):
                        ps = psmm.tile([_P, _NB], dt.float32, tag="mm")
                        for kt in range(_KT):
                            nc.tensor.matmul(
                                ps,
                                lhsT=wk[:, kt, dh * _P:(dh + 1) * _P],
                                rhs=cT[:, kt, jb * _NB:(jb + 1) * _NB],
                                start=(kt == 0), stop=(kt == _KT - 1),
                            )
                        nc.vector.tensor_copy(
                            out=kT[:, dh, jb * _NB:(jb + 1) * _NB], in_=ps
                        )

                # v projection: v[j, d] = ctxT^T @ Wv, mask folded in as a
                # per-j scale on the PSUM->SBUF copy.
                for jt in range(_JT):
                    ps = psmm.tile([_P, _D], dt.float32, tag="mm")
                    for kt in range(_KT):
                        nc.tensor.matmul(
                            ps,
                            lhsT=cT[:, kt, jt * _P:(jt + 1) * _P],
                            rhs=wv[:, kt, :],
                            start=(kt == 0), stop=(kt == _KT - 1),
                        )
                    nc.scalar.activation(
                        out=v[:, jt, :], in_=ps, func=Act.Copy,
                        scale=mask01[:, jt:jt + 1],
                    )

                # x -> x_nat (kept for the residual) and xT
                for g in range(_IT // 4):
                    for h in range(2):
                        r0 = (g * 4 + h * 2) * _P
                        nc.sync.dma_start(
                            out=x_nat[:, g * 4 + h * 2:g * 4 + (h + 1) * 2, :],
                            in_=x_d[r0:r0 + 2 * _P, :].rearrange(
                                "(t p) k -> p t k", p=_P
                            ),
                        )
                    if g == 0:
                        load_weights_late()
                    for tt in range(4):
                        it = g * 4 + tt
                        xbf = stage.tile([_P, _K], dt.bfloat16, tag="xbf",
                                         bufs=4)
                        nc.vector.tensor_copy(out=xbf, in_=x_nat[:, it, :])
                        for kt in range(_KT):
                            tr = pstr.tile([_P, _P], dt.bfloat16, tag="tr")
                            nc.tensor.transpose(
                                tr, xbf[:, kt * _P:(kt + 1) * _P], ident_b
                            )
                            dst = xT[:, kt, it * _P:(it + 1) * _P]
                            if kt % 2 == 0:
                                nc.vector.tensor_copy(
                                    out=dst.bitcast(dt.uint32),
                                    in_=tr[:].bitcast(dt.uint32),
                                )
                            else:
                                nc.scalar.copy(out=dst, in_=tr)
                        # bo rides in the residual; GpSimd is otherwise
                        # idle, and this is off the critical path.
                        nc.gpsimd.tensor_add(
                            out=x_nat[:, it, :], in0=x_nat[:, it, :],
                            in1=bo_bc
                        )

                # qT projection: qT[d, i] = Wq^T @ xT
                for dh in range(_DT):
                    for ib in range(_I // _NB):
                        ps = psmm.tile([_P, _NB], dt.float32, tag="mm")
                        for kt in range(_KT):
                            nc.tensor.matmul(
                                ps,
                                lhsT=wq[:, kt, dh * _P:(dh + 1) * _P],
                                rhs=xT[:, kt, ib * _NB:(ib + 1) * _NB],
                                start=(kt == 0), stop=(kt == _KT - 1),
                            )
                        nc.vector.tensor_copy(
                            out=qT[:, dh, ib * _NB:(ib + 1) * _NB], in_=ps
                        )

                # ---------------- attention main loop ----------------
                # Software pipeline at j-tile granularity: while block b's
                # simT+exp stream through PSUM, the PE interleaves block
                # b-1's PV and denominator matmuls — ScalarE's exp
                # (~612ns/tile) is slower than the sim pair (~426ns), so
                # without the interleave the PE stalls on PSUM recycling.
                aTs = {}
                accs = {}
                pdens = {}

                def start_block(b):
                    aTs[b] = attntp.tile(
                        [_P, _JT, _NB], dt.bfloat16, tag="aT", name=f"aT{b}"
                    )
                    accs[b] = [
                        psacc.tile([_P, _NB], dt.float32, tag="acc",
                                   name=f"acc{b}_{dh}")
                        for dh in range(_DT)
                    ]
                    pdens[b] = pstr.tile([1, _NB], dt.float32, tag="den",
                                         bufs=1, name=f"pden{b}")

                def sim_exp(b, jt):
                    ps = psmm.tile([_P, _NB], dt.float32, tag="mm")
                    for dh in range(_DT):
                        nc.tensor.matmul(
                            ps,
                            lhsT=kT[:, dh, jt * _P:(jt + 1) * _P],
                            rhs=qT[:, dh, b * _NB:(b + 1) * _NB],
                            start=(dh == 0), stop=(dh == _DT - 1),
                        )
                    nc.scalar.activation(
                        out=aTs[b][:, jt, :], in_=ps,
                        func=Act.Exp, bias=0.0, scale=_SCALE,
                    )

                def pv_denom(b, jt):
                    aT = aTs[b]
                    for dh in range(_DT):
                        nc.tensor.matmul(
                            accs[b][dh],
                            lhsT=v[:, jt, dh * _P:(dh + 1) * _P],
                            rhs=aT[:, jt, :],
                            start=(jt == 0), stop=(jt == _JT - 1),
                        )
                    nc.tensor.matmul(
                        pdens[b],
                        lhsT=mask01b[:, jt:jt + 1],
                        rhs=aT[:, jt, :],
                        start=(jt == 0), stop=(jt == _JT - 1),
                    )

                def finish_block(b):
                    for dh in range(_DT):
                        nc.vector.tensor_copy(
                            out=oT[:, dh, b * _NB:(b + 1) * _NB],
                            in_=accs[b][dh],
                        )
                    del accs[b], aTs[b]
                    den_sb = small.tile([1, _NB], dt.float32, tag="densb")
                    nc.vector.tensor_copy(out=den_sb, in_=pdens.pop(b))
                    # Transpose denom to i-major via K=1 matmuls, then 1/x.
                    for tt in range(4):
                        t = b * 4 + tt
                        trd = pstr.tile([_P, 1], dt.float32, tag="tr",
                                        name=f"trd{t}")
                        nc.tensor.matmul(
                            trd,
                            lhsT=den_sb[:, tt * _P:(tt + 1) * _P],
                            rhs=one_one,
                            start=True, stop=True,
                        )
                        nc.vector.reciprocal(out=recips[:, t:t + 1], in_=trd)
                    # Output projection + softmax normalization + residual.
                    for tt in range(4):
                        t = b * 4 + tt
                        yp = psmm.tile([_P, _K], dt.float32, tag="mm")
                        for dh in range(_DT):
                            nc.tensor.matmul(
                                yp,
                                lhsT=oT[:, dh, t * _P:(t + 1) * _P],
                                rhs=wo[:, dh, :],
                                start=(dh == 0), stop=(dh == _DT - 1),
                            )
                        ys = youtp.tile([_P, _K], dt.float32, tag="ys")
                        nc.vector.scalar_tensor_tensor(
                            out=ys, in0=yp, scalar=recips[:, t:t + 1],
                            in1=x_nat[:, t, :], op0=Alu.mult, op1=Alu.add,
                        )
                        nc.sync.dma_start(
                            out=y_d[t * _P:(t + 1) * _P, :], in_=ys
                        )

                n_blocks = _I // _NB
                for b in range(n_blocks):
                    start_block(b)
                    for jt in range(_JT):
                        sim_exp(b, jt)
                        if b > 0:
                            pv_denom(b - 1, jt)
                    if b > 0:
                        finish_block(b - 1)
                for jt in range(_JT):
                    pv_denom(n_blocks - 1, jt)
                finish_block(n_blocks - 1)

    _split_multi_waits(nc, {"*": 1})
    nc.finalize()
    return nc


def kernel(x, context, mask, Wq, Wk, Wv, Wo, bo):
    from concourse.bass_utils import run_bass_kernel_spmd

    if "nc" not in _CACHE:
        _CACHE["nc"] = _build_nc()
    nc = _CACHE["nc"]

    x = np.ascontiguousarray(np.asarray(x, dtype=np.float32))
    context = np.ascontiguousarray(np.asarray(context, dtype=np.float32))
    mask_u8 = np.ascontiguousarray(np.asarray(mask).astype(np.uint8))
    shared = {
        "Wq": np.ascontiguousarray(np.asarray(Wq, dtype=np.float32)),
        "Wk": np.ascontiguousarray(np.asarray(Wk, dtype=np.float32)),
        "Wv": np.ascontiguousarray(np.asarray(Wv, dtype=np.float32)),
        "Wo": np.ascontiguousarray(np.asarray(Wo, dtype=np.float32)),
        "bo": np.ascontiguousarray(np.asarray(bo, dtype=np.float32)),
    }
    in_maps = [
        {"x": x[b], "context": context[b], "mask": mask_u8[b], **shared}
        for b in range(_B)
    ]
    res = run_bass_kernel_spmd(nc, in_maps, core_ids=list(range(_B)))
    return np.stack([res.results[b]["out"] for b in range(_B)], axis=0)



# revision 12
# speedup vs baseline: 1.5647x; 1.5647x over previous
"""Cross-attention Trainium2 kernel (Bass/Tile), SPMD over 8 NeuronCores.

Problem: b=8, i=j=2048, query/context dim 512, inner dim 256.
Sharding: data-parallel over batch - one batch element per core, no
collectives. Each core computes, for its batch element:

    q = x @ Wq ; k = ctx @ Wk ; v = ctx @ Wv
    sim = (q @ k^T) * d^-0.5 ; attn = softmax_j(sim) masked on j
    out = attn @ v ; y = out @ Wo + bo + x

Per-core dataflow (fp8 e4m3 matmuls in DoubleRow perf mode - 2 packed
contraction rows per partition, 0.5 PE cycles per moving row; f32 PSUM):
  1. bo is folded into x on the host (y = attn_out + (x + bo)); the
     residual path stays f32 end to end, so fp8 error only touches the
     attention contribution (~0.6% of output magnitude).
  2. Weights are scaled x16 into fp8 (lifts them out of the e4m3
     subnormal range); compensation is folded into the exp scale, an
     x(1/64) on the attn-out copy, and a x4 mask row used for the
     softmax denominator matmul, so no extra ops are spent on it.
  3. x/ctx stream in, are cast f32->bf16 (Pool), PE-transposed per
     128x128 tile into k-on-partition layout (eight tiles per PSUM
     trq, one packed u16-bitcast DVE copy out), then projected with
     plain bf16 matmuls; the PSUM->SBUF copies of q/k/v quantize to
     x16 fp8 for the attention DoubleRow matmuls.
  4. simT[j,i] is computed TRANSPOSED in one DoubleRow matmul per
     128-j-tile; exp runs on ScalarE over PAIRS of PSUM banks
     ([128,2,512] per instruction) to amortize access latency, writing
     fp8 attn tiles directly.
  5. Softmax denominators ride a DoubleRow matmul against the x4 mask
     pair-row; PV accumulates v^T @ attnT over j in PSUM.  PV for
     blocks 0-1 is DEFERRED until after the input stream so their
     sim+exp can pace the DMA while PSUM accumulator banks double as
     projection scratch.
  6. y = (outT^T @ Wo)*recip + x_nat per 128-i tile, streamed out.
"""

import sys

import numpy as np

if "/opt/trn_rl_repo" not in sys.path:
    sys.path.insert(0, "/opt/trn_rl_repo")

_P = 128          # partitions
_B = 8            # batch == number of cores
_I = 2048         # query sequence length
_J = 2048         # context sequence length
_K = 512          # query/context feature dim
_D = 256          # inner dim
_NB = 512         # matmul free-dim block (one i-block)
_IT = _I // _P    # 16 i tiles
_JT = _J // _P    # 16 j tiles
_NBLK = _I // _NB             # 4 i-blocks
_NPAIR = _JT // 2             # 8 j-tile pairs per block
_WSCALE = 16.0                # weight fp8 pre-scale
_OSCALE = 1.0 / 64.0          # attn-out PSUM->fp8 scale
_MASKVAL = 4.0                # mask row value (denominator scale)
_SCALE_EXP = (float(_D) ** -0.5) / (_WSCALE * _WSCALE)

_CACHE = {}


def _split_multi_waits(nc, limits):
    """Walrus in this container rejects instructions carrying more sem
    waits than its per-template slot count (e.g. Drain allows 1). Move
    excess waits onto wait-only Drain carriers on the same engine,
    inserted just before the instruction - semantically identical."""
    from concourse import mybir

    n_split = 0
    for func in nc.m.functions:
        for block in func.blocks:
            out = []
            for inst in block.instructions:
                si = inst.sync_info
                maxw = limits.get(type(inst).__name__, limits.get("*"))
                if (
                    maxw is not None
                    and si is not None
                    and si.on_wait
                    and len(si.on_wait) > maxw
                ):
                    waits = list(si.on_wait)
                    keep, rest = waits[:maxw], waits[maxw:]
                    for i in range(0, len(rest), 1):
                        car = mybir.InstDrain(
                            name=f"I-waitcar-{nc.next_id()}", ins=[], outs=[]
                        )
                        car.engine = inst.engine
                        car.sync_info = mybir.SyncInfo(
                            on_wait=[rest[i]], on_update=[]
                        )
                        nc.register_instruction(car)
                        out.append(car)
                        n_split += 1
                    inst.sync_info = mybir.SyncInfo(
                        on_wait=keep, on_update=list(si.on_update or [])
                    )
                out.append(inst)
            block.instructions = out
    return n_split


def _build_nc():
    import concourse.bass as bass
    import concourse.tile as tile
    from concourse import mybir
    from concourse.masks import make_identity

    dt = mybir.dt
    Alu = mybir.AluOpType
    Act = mybir.ActivationFunctionType
    DR = mybir.MatmulPerfMode.DoubleRow
    F8 = dt.float8e4

    nc = bass.Bass("TRN2", target_bir_lowering=False)

    x_d = nc.dram_tensor("x", [_I, _K], dt.float32, kind="ExternalInput")
    c_d = nc.dram_tensor("context", [_J, _K], dt.float32, kind="ExternalInput")
    m_d = nc.dram_tensor("mask", [_J], dt.uint8, kind="ExternalInput")
    wq_d = nc.dram_tensor("Wq", [_K, _D], dt.float32, kind="ExternalInput")
    wk_d = nc.dram_tensor("Wk", [_K, _D], dt.float32, kind="ExternalInput")
    wv_d = nc.dram_tensor("Wv", [_K, _D], dt.float32, kind="ExternalInput")
    wo_d = nc.dram_tensor("Wo", [_D, _K], dt.float32, kind="ExternalInput")
    y_d = nc.dram_tensor("out", [_I, _K], dt.float32, kind="ExternalOutput")

    with tile.TileContext(nc) as tc:
        with (
            tc.tile_pool(name="persist", bufs=1) as persist,
            tc.tile_pool(name="stage", bufs=3) as stage,
            tc.tile_pool(name="cast8", bufs=4) as cast8,
            tc.tile_pool(name="small", bufs=4) as small,
            tc.tile_pool(name="aTp", bufs=3) as aTp,
            tc.tile_pool(name="yout", bufs=3) as youtp,
            tc.tile_pool(name="psring", bufs=1, space="PSUM") as psring,
            tc.tile_pool(name="psacc", bufs=2, space="PSUM") as psacc,
            tc.tile_pool(name="psden", bufs=1, space="PSUM") as psden,
            tc.tile_pool(name="psaux", bufs=1, space="PSUM") as psaux,
        ):
            # ---------------- constants ----------------
            ident16 = persist.tile([_P, _P], dt.bfloat16, tag="ident16")
            make_identity(nc, ident16)
            one_one = persist.tile([1, 1], dt.float32, tag="one_one")
            nc.vector.memset(one_one, 1.0)

            # ---------------- persistent activations ----------------
            # k-major fp8 activations: [k0, t, kp, seq] with k = kp*256
            # + t*128 + k0 (t is the DoubleRow slot dim).
            # k-major bf16 activations: [k0, kt, seq], k = kt*128 + k0.
            xT16 = persist.tile([_P, 4, _I], dt.bfloat16, tag="xT16")
            cT16 = persist.tile([_P, 4, _J], dt.bfloat16, tag="cT16")
            # d-major fp8: [d0, dh, seq], d = dh*128 + d0.
            qT8 = persist.tile([_P, 2, _I], F8, tag="qT8")
            kT8 = persist.tile([_P, 2, _J], F8, tag="kT8")
            oT8 = persist.tile([_P, 2, _I], F8, tag="oT8")
            # v: [j0, t, pair, d] with j = pair*256 + t*128 + j0.
            v8 = persist.tile([_P, 2, _NPAIR, _D], F8, tag="v8")
            x_nat = persist.tile([_P, _IT, _K], dt.float32, tag="xnat")
            recips = persist.tile([_P, _IT], dt.float32, tag="recips")

            wq16 = persist.tile([_P, 4, _D], dt.bfloat16, tag="wq16")
            wk16 = persist.tile([_P, 4, _D], dt.bfloat16, tag="wk16")
            wv16 = persist.tile([_P, 4, _D], dt.bfloat16, tag="wv16")
            wo8 = persist.tile([_P, 2, _K], F8, tag="wo8")
            # mask block for the denominator matmul: [j0, t, pair, m]
            # with column m=0 holding _MASKVAL*mask[j] and m>0 zero, so
            # the lhsT keeps a full 128-wide M dim (ISA requirement).
            mask8 = persist.tile([_P, 2, _NPAIR, _P], F8, tag="mask8")
            # per-j-tile scale for the v copy: _WSCALE where unmasked
            maskv = persist.tile([_P, _JT], dt.float32, tag="maskv")

            # PSUM ring for sim tiles: 4 banks, exp reads bank pairs.
            simring = psring.tile([_P, 4, _NB], dt.float32, tag="simring")

            # ---------------- small helpers ----------------
            def load_weight_kmaj(w_dram, w_sb, name):
                ws = stage.tile([_P, 4, _D], dt.float32, tag="wstage",
                                name=f"ws_{name}")
                nc.sync.dma_start(
                    out=ws,
                    in_=w_dram[:].rearrange("(kt p) d -> p kt d", p=_P),
                )
                nc.gpsimd.tensor_copy(out=w_sb, in_=ws)

            def load_mask():
                msk_b = small.tile([_P, _JT], dt.uint8, tag="mskb")
                nc.sync.dma_start(
                    out=msk_b, in_=m_d[:].rearrange("(jt p) -> p jt", p=_P)
                )
                nc.vector.tensor_scalar_mul(out=maskv, in0=msk_b,
                                            scalar1=_WSCALE)
                nc.gpsimd.memset(mask8, 0.0)
                # mask8[p, t, pair, 0] = maskval * mask[jt=2*pair+t]
                mr = msk_b[:].rearrange("p (pair t) -> p t pair", t=2)
                nc.vector.tensor_scalar_mul(
                    out=mask8[:, :, :, 0:1],
                    in0=bass.AP(tensor=mr.tensor, offset=mr.offset,
                                ap=list(mr.ap) + [[1, 1]]),
                    scalar1=_MASKVAL,
                )

            aux_flip = [0]

            def trq_pool():
                # Alternate transpose quads between the two single-bank
                # pools for double buffering; psden is free until the
                # first denominator matmul (post-stream).
                aux_flip[0] ^= 1
                return psaux if aux_flip[0] else psden

            def cast_transpose2(srcs, dstT16, tile0, name):
                """Two [128,512] f32 row-tiles -> bf16 (Pool) -> 8 PE
                transposes into one packed PSUM tile -> one u16 DVE copy
                into dstT16 columns [tile0, tile0+1]."""
                trq = None
                for h, src_f32 in enumerate(srcs):
                    t16 = cast8.tile([_P, _K], dt.bfloat16, tag="t16",
                                     name=f"t16_{name}{h}")
                    nc.gpsimd.tensor_copy(out=t16, in_=src_f32)
                    if trq is None:
                        pool = trq_pool()
                        tag = "aux" if pool is psaux else "den"
                        trq = pool.tile([_P, 2, 4, _P], dt.bfloat16,
                                        tag=tag, name=f"trq_{name}")
                    for kt in range(4):
                        nc.tensor.transpose(
                            trq[:, h, kt, :], t16[:, kt * _P:(kt + 1) * _P],
                            ident16,
                        )
                # trq [128, tile h, kt, 128] -> dstT16 [128, kt, (tile, j)]
                dst = dstT16[:, :, tile0 * _P:(tile0 + 2) * _P]
                dview = dst.bitcast(dt.uint16)
                dstp = bass.AP(
                    tensor=dview.tensor, offset=dview.offset,
                    ap=[dview.ap[0], [dview.ap[1][0], 4], [_P, 2], [1, _P]],
                )
                nc.vector.tensor_copy(
                    out=dstp,
                    in_=trq[:].bitcast(dt.uint16).rearrange(
                        "p h s n -> p s h n"
                    ),
                )

            def kproj(jp):
                """kT8 columns for j-tile pair jp (256 j), x16 fp8."""
                ps = psacc.tile([_P, 2, 2 * _P], dt.float32, tag="acc",
                                name=f"kp{jp}")
                for dh in range(2):
                    for kt in range(4):
                        nc.tensor.matmul(
                            ps[:, dh, :],
                            lhsT=wk16[:, kt, dh * _P:(dh + 1) * _P],
                            rhs=cT16[:, kt, jp * 2 * _P:(jp + 1) * 2 * _P],
                            start=(kt == 0), stop=(kt == 3),
                        )
                nc.vector.tensor_scalar_mul(
                    out=kT8[:, :, jp * 2 * _P:(jp + 1) * 2 * _P], in0=ps,
                    scalar1=_WSCALE,
                )

            def qproj(ib):
                """qT8 columns for i-block ib (512 i), x16 fp8."""
                for dh in range(2):
                    ps = psacc.tile([_P, _NB], dt.float32, tag="acc",
                                    name=f"qp{ib}_{dh}")
                    for kt in range(4):
                        nc.tensor.matmul(
                            ps,
                            lhsT=wq16[:, kt, dh * _P:(dh + 1) * _P],
                            rhs=xT16[:, kt, ib * _NB:(ib + 1) * _NB],
                            start=(kt == 0), stop=(kt == 3),
                        )
                    nc.vector.tensor_scalar_mul(
                        out=qT8[:, dh, ib * _NB:(ib + 1) * _NB], in0=ps,
                        scalar1=_WSCALE,
                    )

            def vproj(jt):
                ps = psacc.tile([_P, _D], dt.float32, tag="acc",
                                name=f"vp{jt}")
                for kt in range(4):
                    nc.tensor.matmul(
                        ps,
                        lhsT=cT16[:, kt, jt * _P:(jt + 1) * _P],
                        rhs=wv16[:, kt, :],
                        start=(kt == 0), stop=(kt == 3),
                    )
                # x16 (in maskv) + mask zeroing ride the fp8 quantize
                nc.vector.tensor_scalar_mul(
                    out=v8[:, jt % 2, jt // 2, :], in0=ps,
                    scalar1=maskv[:, jt:jt + 1],
                )

            # ---------------- sim / exp / PV / finish ----------------
            aTs = {}
            accs = {}
            pdens = {}
            gp_counter = [0]

            def sim_exp_pair(b, p):
                """sim for j-tiles (2p, 2p+1) of block b + one paired exp."""
                gp = gp_counter[0]
                gp_counter[0] += 1
                s0 = (2 * gp) % 4
                for t01 in range(2):
                    jt = 2 * p + t01
                    nc.tensor.matmul(
                        simring[:, s0 + t01, :],
                        lhsT=kT8[:, :, jt * _P:(jt + 1) * _P],
                        rhs=qT8[:, :, b * _NB:(b + 1) * _NB],
                        start=True, stop=True,
                        perf_mode=DR,
                    )
                if b not in aTs:
                    aTs[b] = aTp.tile([_P, _NPAIR, 2, _NB], F8, tag="aT",
                                      name=f"aT{b}")
                nc.scalar.activation(
                    out=aTs[b][:, p, :, :], in_=simring[:, s0:s0 + 2, :],
                    func=Act.Exp, bias=0.0, scale=_SCALE_EXP,
                )

            def pv_den_pair(b, p):
                if b not in accs:
                    accs[b] = [
                        psacc.tile([_P, _NB], dt.float32, tag="acc",
                                   name=f"acc{b}_{dh}")
                        for dh in range(2)
                    ]
                    pdens[b] = psden.tile([_P, _NB], dt.float32, tag="den",
                                          name=f"pden{b}")
                aT = aTs[b]
                for dh in range(2):
                    nc.tensor.matmul(
                        accs[b][dh],
                        lhsT=v8[:, :, p, dh * _P:(dh + 1) * _P],
                        rhs=aT[:, p, :, :],
                        start=(p == 0), stop=(p == _NPAIR - 1),
                        perf_mode=DR,
                    )
                nc.tensor.matmul(
                    pdens[b],
                    lhsT=mask8[:, :, p, :],
                    rhs=aT[:, p, :, :],
                    start=(p == 0), stop=(p == _NPAIR - 1),
                    perf_mode=DR,
                )

            def finish_a(b):
                """denominator transpose + reciprocals + attn-out to fp8."""
                den_sb = small.tile([1, _NB], dt.float32, tag="densb",
                                    name=f"den{b}")
                nc.vector.tensor_copy(out=den_sb, in_=pdens.pop(b)[0:1, :])
                trdq = psaux.tile([_P, 4], dt.float32, tag="aux",
                                  name=f"trdq{b}")
                for tt in range(4):
                    nc.tensor.matmul(
                        trdq[:, tt:tt + 1],
                        lhsT=den_sb[:, tt * _P:(tt + 1) * _P],
                        rhs=one_one,
                        start=(tt == 0), stop=(tt == 3),
                        skip_group_check=True,
                    )
                nc.vector.reciprocal(out=recips[:, b * 4:(b + 1) * 4],
                                     in_=trdq)
                for dh in range(2):
                    nc.scalar.activation(
                        out=oT8[:, dh, b * _NB:(b + 1) * _NB],
                        in_=accs[b][dh], func=Act.Copy, scale=_OSCALE,
                    )
                del accs[b], aTs[b]

            def finish_b(b, tt):
                """output projection + normalize + residual for i-tile."""
                t = b * 4 + tt
                yp = psaux.tile([_P, _K], dt.float32, tag="aux",
                                name=f"yp{t}")
                nc.tensor.matmul(
                    yp,
                    lhsT=oT8[:, :, t * _P:(t + 1) * _P],
                    rhs=wo8,
                    start=True, stop=True,
                    perf_mode=DR,
                )
                ys = youtp.tile([_P, _K], dt.float32, tag="ys")
                nc.vector.scalar_tensor_tensor(
                    out=ys, in0=yp, scalar=recips[:, t:t + 1],
                    in1=x_nat[:, t, :], op0=Alu.mult, op1=Alu.add,
                )
                nc.sync.dma_start(out=y_d[t * _P:(t + 1) * _P, :], in_=ys)

            # ---------------- x-block ingest ----------------
            def x_block(ib):
                """DMA 4 i-tiles of x, transpose, project qT8."""
                for h in range(2):
                    r0 = (ib * 4 + h * 2) * _P
                    nc.sync.dma_start(
                        out=x_nat[:, ib * 4 + h * 2:ib * 4 + (h + 1) * 2, :],
                        in_=x_d[r0:r0 + 2 * _P, :].rearrange(
                            "(t p) k -> p t k", p=_P
                        ),
                    )
                for hh in range(2):
                    it = ib * 4 + hh * 2
                    cast_transpose2(
                        (x_nat[:, it, :], x_nat[:, it + 1, :]),
                        xT16, it, f"x{it}",
                    )
                qproj(ib)

            def ctx_chunk(jp):
                """DMA a j-tile pair of ctx, transpose, kproj (+vproj)."""
                cn = stage.tile([_P, 2, _K], dt.float32, tag="cnat",
                                name=f"cn{jp}")
                r0 = jp * 2 * _P
                nc.sync.dma_start(
                    out=cn,
                    in_=c_d[r0:r0 + 2 * _P, :].rearrange(
                        "(t p) k -> p t k", p=_P
                    ),
                )
                cast_transpose2((cn[:, 0, :], cn[:, 1, :]), cT16,
                                jp * 2, f"c{jp * 2}")
                kproj(jp)

            # ================= emission schedule =================
            # Interleaved so Act (exp) starts as early as possible and
            # never starves; PV for blocks 0-1 is deferred past the
            # stream so psacc banks can serve the projections.
            load_weight_kmaj(wk_d, wk16, "wk")
            load_weight_kmaj(wq_d, wq16, "wq")
            load_mask()
            x_block(0)
            ctx_chunk(0)
            ctx_chunk(1)
            # exp pair order during the stream: earliest-available first.
            sim_exp_pair(0, 0)
            load_weight_kmaj(wv_d, wv16, "wv")
            vproj(0), vproj(1), vproj(2), vproj(3)
            sim_exp_pair(0, 1)
            x_block(1)
            ctx_chunk(2)
            sim_exp_pair(1, 0)
            sim_exp_pair(0, 2)
            vproj(4), vproj(5)
            ctx_chunk(3)
            sim_exp_pair(1, 1)
            sim_exp_pair(0, 3)
            vproj(6), vproj(7)
            x_block(2)
            ctx_chunk(4)
            sim_exp_pair(1, 2)
            sim_exp_pair(0, 4)
            vproj(8), vproj(9)
            ctx_chunk(5)
            sim_exp_pair(1, 3)
            sim_exp_pair(0, 5)
            vproj(10), vproj(11)
            x_block(3)
            ctx_chunk(6)
            sim_exp_pair(1, 4)
            sim_exp_pair(0, 6)
            vproj(12), vproj(13)
            ctx_chunk(7)
            sim_exp_pair(1, 5)
            sim_exp_pair(0, 7)
            vproj(14), vproj(15)
            # Wo + wo8 cast ride behind the last ctx chunk.
            ws_o = stage.tile([_P, 2, _K], dt.float32, tag="wstage",
                              name="ws_wo")
            nc.sync.dma_start(
                out=ws_o, in_=wo_d[:].rearrange("(t p) k -> p t k", p=_P)
            )
            nc.gpsimd.tensor_scalar_mul(out=wo8, in0=ws_o,
                                        scalar1=_WSCALE)
            sim_exp_pair(1, 6)
            sim_exp_pair(1, 7)

            # -------- post-stream: blocks 2,3 exp + all PV/finishes ----
            # PE-order interleave: per exp pair, ~2 deferred PV pairs,
            # with finish work woven in one tile at a time so no engine
            # sees a burst that stalls its in-order queue.
            post_sims = [(2, p) for p in range(_NPAIR)] + \
                        [(3, p) for p in range(_NPAIR)]
            pv_order = [(b, p) for b in range(4) for p in range(_NPAIR)]
            fin_a_at = {8: 0, 16: 1, 24: 2, 32: 3}   # after n PV pairs
            pv_i = 0
            fb_queue = []

            def drain_to(n):
                nonlocal pv_i
                while pv_i < n:
                    b, p = pv_order[pv_i]
                    pv_den_pair(b, p)
                    pv_i += 1
                    if pv_i in fin_a_at:
                        bb = fin_a_at[pv_i]
                        finish_a(bb)
                        fb_queue.extend((bb, tt) for tt in range(4))
                    if fb_queue:
                        finish_b(*fb_queue.pop(0))

            for idx, (b, p) in enumerate(post_sims):
                sim_exp_pair(b, p)
                # cap: only PVs whose exp was emitted >=1 slot ago (plus
                # the fully-buffered blocks 0/1); pace: ~2 per sim pair.
                cap = min(16 + max(0, idx - 1), 2 * (idx + 1), 32)
                drain_to(cap)
            drain_to(32)
            while fb_queue:
                finish_b(*fb_queue.pop(0))

    _split_multi_waits(nc, {"*": 1})
    nc.finalize()
    return nc


def kernel(x, context, mask, Wq, Wk, Wv, Wo, bo):
    from concourse.bass_utils import run_bass_kernel_spmd

    if "nc" not in _CACHE:
        _CACHE["nc"] = _build_nc()
    nc = _CACHE["nc"]

    x = np.asarray(x, dtype=np.float32)
    bo = np.asarray(bo, dtype=np.float32)
    # Fold the output bias into the residual input: y = attn_out + (x + bo)
    x_plus = np.ascontiguousarray(x + bo[None, None, :])
    context = np.ascontiguousarray(np.asarray(context, dtype=np.float32))
    mask_u8 = np.ascontiguousarray(np.asarray(mask).astype(np.uint8))
    shared = {
        "Wq": np.ascontiguousarray(np.asarray(Wq, dtype=np.float32)),
        "Wk": np.ascontiguousarray(np.asarray(Wk, dtype=np.float32)),
        "Wv": np.ascontiguousarray(np.asarray(Wv, dtype=np.float32)),
        "Wo": np.ascontiguousarray(np.asarray(Wo, dtype=np.float32)),
    }
    in_maps = [
        {"x": x_plus[b], "context": context[b], "mask": mask_u8[b], **shared}
        for b in range(_B)
    ]
    res = run_bass_kernel_spmd(nc, in_maps, core_ids=list(range(_B)))
    return np.stack([res.results[b]["out"] for b in range(_B)], axis=0)


# revision 13
# speedup vs baseline: 1.5707x; 1.0038x over previous
"""Cross-attention Trainium2 kernel (Bass/Tile), SPMD over 8 NeuronCores.

Problem: b=8, i=j=2048, query/context dim 512, inner dim 256.
Sharding: data-parallel over batch - one batch element per core, no
collectives. Each core computes, for its batch element:

    q = x @ Wq ; k = ctx @ Wk ; v = ctx @ Wv
    sim = (q @ k^T) * d^-0.5 ; attn = softmax_j(sim) masked on j
    out = attn @ v ; y = out @ Wo + bo + x

Per-core dataflow (fp8 e4m3 matmuls in DoubleRow perf mode - 2 packed
contraction rows per partition, 0.5 PE cycles per moving row; f32 PSUM):
  1. bo is folded into x on the host (y = attn_out + (x + bo)); the
     residual path stays f32 end to end, so fp8 error only touches the
     attention contribution (~0.6% of output magnitude).
  2. Weights are scaled x16 into fp8 (lifts them out of the e4m3
     subnormal range); compensation is folded into the exp scale, an
     x(1/64) on the attn-out copy, and a x4 mask row used for the
     softmax denominator matmul, so no extra ops are spent on it.
  3. x/ctx stream in, are cast f32->bf16 (Pool), PE-transposed per
     128x128 tile into k-on-partition layout (eight tiles per PSUM
     trq, one packed u16-bitcast DVE copy out), then projected with
     plain bf16 matmuls; the PSUM->SBUF copies of q/k/v quantize to
     x16 fp8 for the attention DoubleRow matmuls.
  4. simT[j,i] is computed TRANSPOSED in one DoubleRow matmul per
     128-j-tile; exp runs on ScalarE over PAIRS of PSUM banks
     ([128,2,512] per instruction) to amortize access latency, writing
     fp8 attn tiles directly.
  5. Softmax denominators ride a DoubleRow matmul against the x4 mask
     pair-row; PV accumulates v^T @ attnT over j in PSUM.  PV for
     blocks 0-1 is DEFERRED until after the input stream so their
     sim+exp can pace the DMA while PSUM accumulator banks double as
     projection scratch.
  6. y = (outT^T @ Wo)*recip + x_nat per 128-i tile, streamed out.
"""

import sys

import numpy as np

if "/opt/trn_rl_repo" not in sys.path:
    sys.path.insert(0, "/opt/trn_rl_repo")

_P = 128          # partitions
_B = 8            # batch == number of cores
_I = 2048         # query sequence length
_J = 2048         # context sequence length
_K = 512          # query/context feature dim
_D = 256          # inner dim
_NB = 512         # matmul free-dim block (one i-block)
_IT = _I // _P    # 16 i tiles
_JT = _J // _P    # 16 j tiles
_NBLK = _I // _NB             # 4 i-blocks
_NPAIR = _JT // 2             # 8 j-tile pairs per block
_WSCALE = 16.0                # weight fp8 pre-scale
_OSCALE = 1.0 / 64.0          # attn-out PSUM->fp8 scale
_MASKVAL = 4.0                # mask row value (denominator scale)
_SCALE_EXP = (float(_D) ** -0.5) / (_WSCALE * _WSCALE)

_CACHE = {}


def _split_multi_waits(nc, limits):
    """Walrus in this container rejects instructions carrying more sem
    waits than its per-template slot count (e.g. Drain allows 1). Move
    excess waits onto wait-only Drain carriers on the same engine,
    inserted just before the instruction - semantically identical."""
    from concourse import mybir

    n_split = 0
    for func in nc.m.functions:
        for block in func.blocks:
            out = []
            for inst in block.instructions:
                si = inst.sync_info
                maxw = limits.get(type(inst).__name__, limits.get("*"))
                if (
                    maxw is not None
                    and si is not None
                    and si.on_wait
                    and len(si.on_wait) > maxw
                ):
                    waits = list(si.on_wait)
                    keep, rest = waits[:maxw], waits[maxw:]
                    for i in range(0, len(rest), 1):
                        car = mybir.InstDrain(
                            name=f"I-waitcar-{nc.next_id()}", ins=[], outs=[]
                        )
                        car.engine = inst.engine
                        car.sync_info = mybir.SyncInfo(
                            on_wait=[rest[i]], on_update=[]
                        )
                        nc.register_instruction(car)
                        out.append(car)
                        n_split += 1
                    inst.sync_info = mybir.SyncInfo(
                        on_wait=keep, on_update=list(si.on_update or [])
                    )
                out.append(inst)
            block.instructions = out
    return n_split


def _build_nc():
    import concourse.bass as bass
    import concourse.tile as tile
    from concourse import mybir
    from concourse.masks import make_identity

    dt = mybir.dt
    Alu = mybir.AluOpType
    Act = mybir.ActivationFunctionType
    DR = mybir.MatmulPerfMode.DoubleRow
    F8 = dt.float8e4

    nc = bass.Bass("TRN2", target_bir_lowering=False)

    x_d = nc.dram_tensor("x", [_I, _K], dt.float32, kind="ExternalInput")
    c_d = nc.dram_tensor("context", [_J, _K], dt.float32, kind="ExternalInput")
    m_d = nc.dram_tensor("mask", [_J], dt.uint8, kind="ExternalInput")
    wq_d = nc.dram_tensor("Wq", [_K, _D], dt.float32, kind="ExternalInput")
    wk_d = nc.dram_tensor("Wk", [_K, _D], dt.float32, kind="ExternalInput")
    wv_d = nc.dram_tensor("Wv", [_K, _D], dt.float32, kind="ExternalInput")
    wo_d = nc.dram_tensor("Wo", [_D, _K], dt.float32, kind="ExternalInput")
    y_d = nc.dram_tensor("out", [_I, _K], dt.float32, kind="ExternalOutput")

    with tile.TileContext(nc) as tc:
        with (
            tc.tile_pool(name="persist", bufs=1) as persist,
            tc.tile_pool(name="stage", bufs=3) as stage,
            tc.tile_pool(name="cast8", bufs=4) as cast8,
            tc.tile_pool(name="small", bufs=4) as small,
            tc.tile_pool(name="aTp", bufs=4) as aTp,
            tc.tile_pool(name="yout", bufs=3) as youtp,
            tc.tile_pool(name="psring", bufs=1, space="PSUM") as psring,
            tc.tile_pool(name="psacc", bufs=2, space="PSUM") as psacc,
            tc.tile_pool(name="psden", bufs=1, space="PSUM") as psden,
            tc.tile_pool(name="psaux", bufs=1, space="PSUM") as psaux,
        ):
            # ---------------- constants ----------------
            ident16 = persist.tile([_P, _P], dt.bfloat16, tag="ident16")
            make_identity(nc, ident16)
            one_one = persist.tile([1, 1], dt.float32, tag="one_one")
            nc.vector.memset(one_one, 1.0)

            # ---------------- persistent activations ----------------
            # k-major fp8 activations: [k0, t, kp, seq] with k = kp*256
            # + t*128 + k0 (t is the DoubleRow slot dim).
            # k-major bf16 activations: [k0, kt, seq], k = kt*128 + k0.
            xT16 = persist.tile([_P, 4, _I], dt.bfloat16, tag="xT16")
            cT16 = persist.tile([_P, 4, _J], dt.bfloat16, tag="cT16")
            # d-major fp8: [d0, dh, seq], d = dh*128 + d0.
            qT8 = persist.tile([_P, 2, _I], F8, tag="qT8")
            kT8 = persist.tile([_P, 2, _J], F8, tag="kT8")
            oT8 = persist.tile([_P, 2, _I], F8, tag="oT8")
            # v: [j0, t, pair, d] with j = pair*256 + t*128 + j0.
            v8 = persist.tile([_P, 2, _NPAIR, _D], F8, tag="v8")
            x_nat = persist.tile([_P, _IT, _K], dt.float32, tag="xnat")
            recips = persist.tile([_P, _IT], dt.float32, tag="recips")

            wq16 = persist.tile([_P, 4, _D], dt.bfloat16, tag="wq16")
            wk16 = persist.tile([_P, 4, _D], dt.bfloat16, tag="wk16")
            wv16 = persist.tile([_P, 4, _D], dt.bfloat16, tag="wv16")
            wo8 = persist.tile([_P, 2, _K], F8, tag="wo8")
            # mask block for the denominator matmul: [j0, t, pair, m]
            # with column m=0 holding _MASKVAL*mask[j] and m>0 zero, so
            # the lhsT keeps a full 128-wide M dim (ISA requirement).
            mask8 = persist.tile([_P, 2, _NPAIR, _P], F8, tag="mask8")
            # per-j-tile scale for the v copy: _WSCALE where unmasked
            maskv = persist.tile([_P, _JT], dt.float32, tag="maskv")

            # PSUM ring for sim tiles: 4 banks, exp reads bank pairs.
            simring = psring.tile([_P, 4, _NB], dt.float32, tag="simring")

            # ---------------- small helpers ----------------
            def load_weight_kmaj(w_dram, w_sb, name):
                ws = stage.tile([_P, 4, _D], dt.float32, tag="wstage",
                                name=f"ws_{name}")
                nc.sync.dma_start(
                    out=ws,
                    in_=w_dram[:].rearrange("(kt p) d -> p kt d", p=_P),
                )
                nc.vector.tensor_copy(out=w_sb, in_=ws)

            def load_mask():
                msk_b = small.tile([_P, _JT], dt.uint8, tag="mskb")
                nc.sync.dma_start(
                    out=msk_b, in_=m_d[:].rearrange("(jt p) -> p jt", p=_P)
                )
                nc.vector.tensor_scalar_mul(out=maskv, in0=msk_b,
                                            scalar1=_WSCALE)
                nc.gpsimd.memset(mask8, 0.0)
                # mask8[p, t, pair, 0] = maskval * mask[jt=2*pair+t]
                mr = msk_b[:].rearrange("p (pair t) -> p t pair", t=2)
                nc.vector.tensor_scalar_mul(
                    out=mask8[:, :, :, 0:1],
                    in0=bass.AP(tensor=mr.tensor, offset=mr.offset,
                                ap=list(mr.ap) + [[1, 1]]),
                    scalar1=_MASKVAL,
                )

            aux_flip = [0]

            def trq_pool():
                # Alternate transpose quads between the two single-bank
                # pools for double buffering; psden is free until the
                # first denominator matmul (post-stream).
                aux_flip[0] ^= 1
                return psaux if aux_flip[0] else psden

            def cast_transpose2(srcs, dstT16, tile0, name, cast_eng=None):
                """Two [128,512] f32 row-tiles -> bf16 -> 8 PE
                transposes into one packed PSUM tile -> one u16 DVE copy
                into dstT16 columns [tile0, tile0+1]."""
                trq = None
                for h, src_f32 in enumerate(srcs):
                    t16 = cast8.tile([_P, _K], dt.bfloat16, tag="t16",
                                     name=f"t16_{name}{h}")
                    (cast_eng or nc.gpsimd).tensor_copy(out=t16, in_=src_f32)
                    if trq is None:
                        pool = trq_pool()
                        tag = "aux" if pool is psaux else "den"
                        trq = pool.tile([_P, 2, 4, _P], dt.bfloat16,
                                        tag=tag, name=f"trq_{name}")
                    for kt in range(4):
                        nc.tensor.transpose(
                            trq[:, h, kt, :], t16[:, kt * _P:(kt + 1) * _P],
                            ident16,
                        )
                # trq [128, tile h, kt, 128] -> dstT16 [128, kt, (tile, j)]
                dst = dstT16[:, :, tile0 * _P:(tile0 + 2) * _P]
                dview = dst.bitcast(dt.uint16)
                dstp = bass.AP(
                    tensor=dview.tensor, offset=dview.offset,
                    ap=[dview.ap[0], [dview.ap[1][0], 4], [_P, 2], [1, _P]],
                )
                nc.vector.tensor_copy(
                    out=dstp,
                    in_=trq[:].bitcast(dt.uint16).rearrange(
                        "p h s n -> p s h n"
                    ),
                )

            def kproj(jp):
                """kT8 columns for j-tile pair jp (256 j), x16 fp8."""
                ps = psacc.tile([_P, 2, 2 * _P], dt.float32, tag="acc",
                                name=f"kp{jp}")
                for dh in range(2):
                    for kt in range(4):
                        nc.tensor.matmul(
                            ps[:, dh, :],
                            lhsT=wk16[:, kt, dh * _P:(dh + 1) * _P],
                            rhs=cT16[:, kt, jp * 2 * _P:(jp + 1) * 2 * _P],
                            start=(kt == 0), stop=(kt == 3),
                        )
                nc.vector.tensor_scalar_mul(
                    out=kT8[:, :, jp * 2 * _P:(jp + 1) * 2 * _P], in0=ps,
                    scalar1=_WSCALE,
                )

            def qproj(ib):
                """qT8 columns for i-block ib (512 i), x16 fp8."""
                for dh in range(2):
                    ps = psacc.tile([_P, _NB], dt.float32, tag="acc",
                                    name=f"qp{ib}_{dh}")
                    for kt in range(4):
                        nc.tensor.matmul(
                            ps,
                            lhsT=wq16[:, kt, dh * _P:(dh + 1) * _P],
                            rhs=xT16[:, kt, ib * _NB:(ib + 1) * _NB],
                            start=(kt == 0), stop=(kt == 3),
                        )
                    nc.vector.tensor_scalar_mul(
                        out=qT8[:, dh, ib * _NB:(ib + 1) * _NB], in0=ps,
                        scalar1=_WSCALE,
                    )

            def vproj(jt):
                ps = psacc.tile([_P, _D], dt.float32, tag="acc",
                                name=f"vp{jt}")
                for kt in range(4):
                    nc.tensor.matmul(
                        ps,
                        lhsT=cT16[:, kt, jt * _P:(jt + 1) * _P],
                        rhs=wv16[:, kt, :],
                        start=(kt == 0), stop=(kt == 3),
                    )
                # x16 (in maskv) + mask zeroing ride the fp8 quantize
                nc.scalar.activation(
                    out=v8[:, jt % 2, jt // 2, :], in_=ps, func=Act.Copy,
                    scale=maskv[:, jt:jt + 1],
                )

            # ---------------- sim / exp / PV / finish ----------------
            aTs = {}
            accs = {}
            pdens = {}
            gp_counter = [0]

            def sim_exp_pair(b, p):
                """sim for j-tiles (2p, 2p+1) of block b + one paired exp."""
                gp = gp_counter[0]
                gp_counter[0] += 1
                s0 = (2 * gp) % 4
                for t01 in range(2):
                    jt = 2 * p + t01
                    nc.tensor.matmul(
                        simring[:, s0 + t01, :],
                        lhsT=kT8[:, :, jt * _P:(jt + 1) * _P],
                        rhs=qT8[:, :, b * _NB:(b + 1) * _NB],
                        start=True, stop=True,
                        perf_mode=DR,
                    )
                if b not in aTs:
                    aTs[b] = aTp.tile([_P, _NPAIR, 2, _NB], F8, tag="aT",
                                      name=f"aT{b}")
                nc.scalar.activation(
                    out=aTs[b][:, p, :, :], in_=simring[:, s0:s0 + 2, :],
                    func=Act.Exp, bias=0.0, scale=_SCALE_EXP,
                )

            def pv_den_pair(b, p):
                if b not in accs:
                    accs[b] = [
                        psacc.tile([_P, _NB], dt.float32, tag="acc",
                                   name=f"acc{b}_{dh}")
                        for dh in range(2)
                    ]
                    pdens[b] = psden.tile([_P, _NB], dt.float32, tag="den",
                                          name=f"pden{b}")
                aT = aTs[b]
                for dh in range(2):
                    nc.tensor.matmul(
                        accs[b][dh],
                        lhsT=v8[:, :, p, dh * _P:(dh + 1) * _P],
                        rhs=aT[:, p, :, :],
                        start=(p == 0), stop=(p == _NPAIR - 1),
                        perf_mode=DR,
                    )
                nc.tensor.matmul(
                    pdens[b],
                    lhsT=mask8[:, :, p, :],
                    rhs=aT[:, p, :, :],
                    start=(p == 0), stop=(p == _NPAIR - 1),
                    perf_mode=DR,
                )

            def finish_a(b):
                """denominator transpose + reciprocals + attn-out to fp8."""
                den_sb = small.tile([1, _NB], dt.float32, tag="densb",
                                    name=f"den{b}")
                nc.vector.tensor_copy(out=den_sb, in_=pdens.pop(b)[0:1, :])
                trdq = psaux.tile([_P, 4], dt.float32, tag="aux",
                                  name=f"trdq{b}")
                for tt in range(4):
                    nc.tensor.matmul(
                        trdq[:, tt:tt + 1],
                        lhsT=den_sb[:, tt * _P:(tt + 1) * _P],
                        rhs=one_one,
                        start=(tt == 0), stop=(tt == 3),
                        skip_group_check=True,
                    )
                nc.vector.reciprocal(out=recips[:, b * 4:(b + 1) * 4],
                                     in_=trdq)
                for dh in range(2):
                    nc.vector.tensor_scalar_mul(
                        out=oT8[:, dh, b * _NB:(b + 1) * _NB],
                        in0=accs[b][dh], scalar1=_OSCALE,
                    )
                del accs[b], aTs[b]

            def finish_b(b, tt):
                """output projection + normalize + residual for i-tile."""
                t = b * 4 + tt
                yp = psaux.tile([_P, _K], dt.float32, tag="aux",
                                name=f"yp{t}")
                nc.tensor.matmul(
                    yp,
                    lhsT=oT8[:, :, t * _P:(t + 1) * _P],
                    rhs=wo8,
                    start=True, stop=True,
                    perf_mode=DR,
                )
                ys = youtp.tile([_P, _K], dt.float32, tag="ys")
                nc.vector.scalar_tensor_tensor(
                    out=ys, in0=yp, scalar=recips[:, t:t + 1],
                    in1=x_nat[:, t, :], op0=Alu.mult, op1=Alu.add,
                )
                nc.sync.dma_start(out=y_d[t * _P:(t + 1) * _P, :], in_=ys)

            # ---------------- x-block ingest ----------------
            def x_block(ib):
                """DMA 4 i-tiles of x, transpose, project qT8."""
                for h in range(2):
                    r0 = (ib * 4 + h * 2) * _P
                    nc.sync.dma_start(
                        out=x_nat[:, ib * 4 + h * 2:ib * 4 + (h + 1) * 2, :],
                        in_=x_d[r0:r0 + 2 * _P, :].rearrange(
                            "(t p) k -> p t k", p=_P
                        ),
                    )
                for hh in range(2):
                    it = ib * 4 + hh * 2
                    cast_transpose2(
                        (x_nat[:, it, :], x_nat[:, it + 1, :]),
                        xT16, it, f"x{it}", cast_eng=nc.vector,
                    )
                qproj(ib)

            def ctx_chunk(jp):
                """DMA a j-tile pair of ctx, transpose, kproj (+vproj)."""
                cn = stage.tile([_P, 2, _K], dt.float32, tag="cnat",
                                name=f"cn{jp}")
                r0 = jp * 2 * _P
                nc.sync.dma_start(
                    out=cn,
                    in_=c_d[r0:r0 + 2 * _P, :].rearrange(
                        "(t p) k -> p t k", p=_P
                    ),
                )
                cast_transpose2((cn[:, 0, :], cn[:, 1, :]), cT16,
                                jp * 2, f"c{jp * 2}")
                kproj(jp)

            # ================= emission schedule =================
            # Interleaved so Act (exp) starts as early as possible and
            # never starves; PV for blocks 0-1 is deferred past the
            # stream so psacc banks can serve the projections.
            load_weight_kmaj(wk_d, wk16, "wk")
            load_weight_kmaj(wq_d, wq16, "wq")
            load_mask()
            x_block(0)
            ctx_chunk(0)
            ctx_chunk(1)
            # exp pair order during the stream: earliest-available first.
            sim_exp_pair(0, 0)
            load_weight_kmaj(wv_d, wv16, "wv")
            vproj(0), vproj(1), vproj(2), vproj(3)
            sim_exp_pair(0, 1)
            x_block(1)
            ctx_chunk(2)
            sim_exp_pair(1, 0)
            sim_exp_pair(0, 2)
            vproj(4), vproj(5)
            ctx_chunk(3)
            sim_exp_pair(1, 1)
            sim_exp_pair(0, 3)
            vproj(6), vproj(7)
            x_block(2)
            ctx_chunk(4)
            sim_exp_pair(1, 2)
            sim_exp_pair(0, 4)
            vproj(8), vproj(9)
            ctx_chunk(5)
            sim_exp_pair(1, 3)
            sim_exp_pair(0, 5)
            vproj(10), vproj(11)
            x_block(3)
            ctx_chunk(6)
            sim_exp_pair(1, 4)
            sim_exp_pair(0, 6)
            vproj(12), vproj(13)
            ctx_chunk(7)
            sim_exp_pair(1, 5)
            sim_exp_pair(0, 7)
            vproj(14), vproj(15)
            # Wo + wo8 cast ride behind the last ctx chunk.
            ws_o = stage.tile([_P, 2, _K], dt.float32, tag="wstage",
                              name="ws_wo")
            nc.sync.dma_start(
                out=ws_o, in_=wo_d[:].rearrange("(t p) k -> p t k", p=_P)
            )
            nc.gpsimd.tensor_scalar_mul(out=wo8, in0=ws_o,
                                        scalar1=_WSCALE)
            sim_exp_pair(1, 6)
            sim_exp_pair(1, 7)

            # -------- post-stream: blocks 2,3 exp + all PV/finishes ----
            # PE-order interleave: per exp pair, ~2 deferred PV pairs,
            # with finish work woven in one tile at a time so no engine
            # sees a burst that stalls its in-order queue.
            post_sims = [(2, p) for p in range(_NPAIR)] + \
                        [(3, p) for p in range(_NPAIR)]
            pv_order = [(b, p) for b in range(4) for p in range(_NPAIR)]
            fin_a_at = {8: 0, 16: 1, 24: 2, 32: 3}   # after n PV pairs
            pv_i = 0
            fb_queue = []

            def drain_to(n):
                nonlocal pv_i
                while pv_i < n:
                    b, p = pv_order[pv_i]
                    pv_den_pair(b, p)
                    pv_i += 1
                    if pv_i in fin_a_at:
                        bb = fin_a_at[pv_i]
                        finish_a(bb)
                        fb_queue.extend((bb, tt) for tt in range(4))
                    if fb_queue:
                        finish_b(*fb_queue.pop(0))

            for idx, (b, p) in enumerate(post_sims):
                sim_exp_pair(b, p)
                # cap: only PVs whose exp was emitted >=1 slot ago (plus
                # the fully-buffered blocks 0/1); pace: ~2 per sim pair.
                cap = min(16 + max(0, idx - 1), 2 * (idx + 1), 32)
                drain_to(cap)
            drain_to(32)
            while fb_queue:
                finish_b(*fb_queue.pop(0))

    _split_multi_waits(nc, {"*": 1})
    nc.finalize()
    return nc


def kernel(x, context, mask, Wq, Wk, Wv, Wo, bo):
    from concourse.bass_utils import run_bass_kernel_spmd

    if "nc" not in _CACHE:
        _CACHE["nc"] = _build_nc()
    nc = _CACHE["nc"]

    x = np.asarray(x, dtype=np.float32)
    bo = np.asarray(bo, dtype=np.float32)
    # Fold the output bias into the residual input: y = attn_out + (x + bo)
    x_plus = np.ascontiguousarray(x + bo[None, None, :])
    context = np.ascontiguousarray(np.asarray(context, dtype=np.float32))
    mask_u8 = np.ascontiguousarray(np.asarray(mask).astype(np.uint8))
    shared = {
        "Wq": np.ascontiguousarray(np.asarray(Wq, dtype=np.float32)),
        "Wk": np.ascontiguousarray(np.asarray(Wk, dtype=np.float32)),
        "Wv": np.ascontiguousarray(np.asarray(Wv, dtype=np.float32)),
        "Wo": np.ascontiguousarray(np.asarray(Wo, dtype=np.float32)),
    }
    in_maps = [
        {"x": x_plus[b], "context": context[b], "mask": mask_u8[b], **shared}
        for b in range(_B)
    ]
    res = run_bass_kernel_spmd(nc, in_maps, core_ids=list(range(_B)))
    return np.stack([res.results[b]["out"] for b in range(_B)], axis=0)
